# revision 1
# baseline (speedup 1.0000x reference)
"""Trainium2 Bass kernel for nn_LutLayer (B=512, depth=4096, SIX=6).

Math: per element with x = inputs[b, d, :] (6 values),
    out = sum_{i=0}^{63} w_i * prod_j q_{j, bit_j(i)},
    q_{j,1} = (1-x_j)+eps, q_{j,0} = x_j+eps,  w_i = g(count0(i)),
    g(c) = logit(clamp(c/6)).

Since w depends only on popcount, out = sum_c g_c * S_c where S_c are the
Poisson-binomial coefficients of prod_j (v_j + u_j t).  The sequence g_c is
annihilated by a palindromic quartic with a double root at t=1 and the
reciprocal pair {tau, 1/tau}, giving the exact closed form

    out = C0 + C1 * sum_j y_j + S3 * [prod_j (y_j + D0) - prod_j (y_j - D0)]

with y_j = 2 x_j - 1 and D0 = (1+2eps)(1+tau)/(tau-1).  Only two 6-factor
products and one 6-term sum per element remain; |S3|^(1/6) is folded into
the affine factors so all intermediates stay O(1) in fp32.

Sharding: data-parallel over batch, 64 batches per core on 8 cores.
"""

import sys
from contextlib import ExitStack

import numpy as np

if "/opt/trn_rl_repo" not in sys.path:
    sys.path.insert(0, "/opt/trn_rl_repo")

import concourse.bass as bass
import concourse.tile as tile
from concourse import mybir
from concourse.bass_utils import run_bass_kernel_spmd

N_CORES = 8
B, DEPTH, SIX = 512, 4096, 6
PER_CORE_B = B // N_CORES            # 64
N_ELEM = PER_CORE_B * DEPTH          # 262144 elements per core
P = 128                              # SBUF partitions
FD_TOT = N_ELEM // P                 # 2048 elements per partition
CHUNK = 512                          # elements per partition per chunk
N_CHUNKS = FD_TOT // CHUNK           # 4

# exact decomposition constants (fp64, derived offline; see module docstring)
D0 = 1.244957288028531
S3 = 0.020370985329978712
C1 = 0.33123508857995426
C0 = 1.0089040713978648e-11
W = S3 ** (1.0 / 6.0)                # folded branch weight, 0.52259911...

SCALE_F = float(2.0 * W)             # scale for both product branches
BIAS_P = float(W * (D0 - 1.0))       # bias for (y + D0) branch
BIAS_N = float(W * (-D0 - 1.0))      # bias for (y - D0) branch
LIN_SCALE = float(2.0 * C1)          # applied to sum_j x_j
LIN_BIAS = float(C0 - 6.0 * C1)      # C0 + C1 * (-6)
# linear branch is computed from sum_j F3_j = SCALE_F*sum_j x_j + 6*BIAS_P
# (keeps the input tile's readers on a single engine for sem-wait limits)
LIN_SCALE2 = float(LIN_SCALE / SCALE_F)
LIN_BIAS2 = float(LIN_BIAS - 6.0 * BIAS_P * LIN_SCALE / SCALE_F)

F32 = mybir.dt.float32

# walrus codegen caps sync-wait commands per instruction (empirically: 1 for
# DMACopy and Pool/GPSIMD ops, 2 for ACT/DVE compute).  Tile's sem assignment
# can exceed that, so excess waits are split onto a standalone EventSemaphore
# on the same engine queue (program order makes that equivalent; the final
# all-engine barrier already uses 15-wait EventSemaphores, so they're legal).
_SPLIT_SKIP = {"InstEventSemaphore", "InstUnconditionalBranch",
               "InstCall", "InstRegisterMove"}


def _wait_budget(inst):
    # Empirically every compute/DMA instruction struct accepts only ONE
    # sync-wait command (EventSemaphore accepts two).
    return 1


def _split_sync_waits(nc):
    for f in nc.m.functions:
        for b in f.blocks:
            new_insts = []
            for inst in b.instructions:
                si = inst.sync_info
                waits = list(si.on_wait) if si and si.on_wait else []
                budget = _wait_budget(inst)
                if type(inst).__name__ not in _SPLIT_SKIP and len(waits) > budget:
                    excess, keep = waits[:-budget], waits[-budget:]
                    for i in range(0, len(excess), 2):  # EventSemaphore: <=2 waits
                        ev = mybir.InstEventSemaphore(
                            name=f"{inst.name}-ws{i}",
                            opcode="EventSemaphore",
                            engine=inst.engine,
                            ins=[],
                            outs=[],
                            sync_info=mybir.SyncInfo(on_wait=excess[i:i + 2],
                                                     on_update=[]),
                            bass_nofuse=True,
                        )
                        new_insts.append(ev)
                    inst.sync_info = mybir.SyncInfo(on_wait=keep,
                                                    on_update=si.on_update)
                new_insts.append(inst)
            b.instructions = new_insts


def _build_bass(chunk=CHUNK, fp_bufs=2, o1_act=False, o2_pool=False, o3_pool=False,
                chunks=None, v4p4_pool=False, merged=False, accum=False):
    if chunks is None:
        chunks = [chunk] * (FD_TOT // chunk)
    assert sum(chunks) == FD_TOT, chunks
    n_chunks = len(chunks)
    nc = bass.Bass()
    x_in = nc.declare_dram_parameter("x", [P, FD_TOT * SIX], F32, isOutput=False)
    y_out = nc.declare_dram_parameter("out", [P, FD_TOT], F32, isOutput=True)

    with tile.TileContext(nc) as tc, ExitStack() as ctx:
        # Sync-wait budgets (walrus codegen): DMACopy and Pool(GPSIMD)
        # instructions tolerate only ONE wait command; ACT handles >=3.
        # So every tile written by DMA or read/written by GPSIMD gets one
        # buffer per chunk (no WAR waits at all), DVE-internal tiles are
        # bufs=1 (same-engine deps need no semaphores), and the remaining
        # multi-wait pressure (slot reuse of F3/F4) lands on ACT.
        xp = ctx.enter_context(tc.tile_pool(name="x", bufs=1))
        fp = ctx.enter_context(tc.tile_pool(name="fct", bufs=fp_bufs))
        tp = ctx.enter_context(tc.tile_pool(name="lvl1", bufs=1))
        vp = ctx.enter_context(tc.tile_pool(name="lvl23", bufs=1))
        s1p = ctx.enter_context(tc.tile_pool(name="sum1", bufs=1))
        slp = ctx.enter_context(tc.tile_pool(name="sum23", bufs=1))
        op = ctx.enter_context(tc.tile_pool(name="out", bufs=1))
        off = 0
        for t in range(n_chunks):
            chunk = chunks[t]
            X = xp.tile([P, chunk * SIX], F32, tag=f"x{t}")
            nc.sync.dma_start(X[:], x_in[:, off * SIX:(off + chunk) * SIX])
            Xv = X[:].rearrange("p (f s) -> p f s", s=SIX)

            if merged:
                # both product branches in one double-width pipeline
                FF = fp.tile([P, 2 * chunk * SIX], F32, tag="ff")
                nc.scalar.activation(FF[:, 0:chunk * SIX], X[:],
                                     mybir.ActivationFunctionType.Copy,
                                     bias=BIAS_P, scale=SCALE_F)
                nc.scalar.activation(FF[:, chunk * SIX:], X[:],
                                     mybir.ActivationFunctionType.Copy,
                                     bias=BIAS_N, scale=SCALE_F)
                FFv = FF[:].rearrange("p (b c s) -> p b c s", b=2, s=SIX)
                TT = tp.tile([P, 2 * chunk * 3], F32, tag="tt")
                TTw = TT[:].rearrange("p (b k c) -> p b c k", b=2, k=3)
                nc.vector.tensor_tensor(TTw, FFv[:, :, :, 0:3], FFv[:, :, :, 3:6],
                                        mybir.AluOpType.mult)
                TTv = TT[:].rearrange("p (b k c) -> p b k c", b=2, k=3)
                VV = vp.tile([P, 2 * chunk], F32, tag="vv")
                VVv = VV[:].rearrange("p (b c) -> p b c", b=2)
                nc.vector.tensor_tensor(VVv, TTv[:, :, 0, :], TTv[:, :, 1, :],
                                        mybir.AluOpType.mult)
                PP2 = vp.tile([P, 2 * chunk], F32, tag="pp2")
                PP2v = PP2[:].rearrange("p (b c) -> p b c", b=2)
                nc.vector.tensor_tensor(PP2v, VVv, TTv[:, :, 2, :],
                                        mybir.AluOpType.mult)
                PP = vp.tile([P, chunk], F32, tag="ppd")
                nc.vector.tensor_tensor(PP[:], PP2[:, 0:chunk], PP2[:, chunk:],
                                        mybir.AluOpType.subtract)
                # linear branch on gpsimd
                S1 = s1p.tile([P, chunk * 3], F32, tag=f"s1_{t}")
                S1w = S1[:].rearrange("p (s f) -> p f s", s=3)
                nc.gpsimd.tensor_tensor(S1w, Xv[:, :, 0:3], Xv[:, :, 3:6],
                                        mybir.AluOpType.add)
                S2 = slp.tile([P, chunk], F32, tag=f"s2_{t}")
                nc.gpsimd.tensor_tensor(S2[:], S1[:, 0:chunk], S1[:, chunk:2 * chunk],
                                        mybir.AluOpType.add)
                L = slp.tile([P, chunk], F32, tag=f"lsum{t}")
                nc.gpsimd.tensor_tensor(L[:], S2[:], S1[:, 2 * chunk:3 * chunk],
                                        mybir.AluOpType.add)
                O1 = op.tile([P, chunk], F32, tag=f"o1_{t}")
                if o1_act:
                    nc.scalar.activation(O1[:], L[:],
                                         mybir.ActivationFunctionType.Copy,
                                         bias=LIN_BIAS, scale=LIN_SCALE)
                else:
                    nc.vector.tensor_scalar(O1[:], L[:], LIN_SCALE, LIN_BIAS,
                                            mybir.AluOpType.mult,
                                            mybir.AluOpType.add)
                O3 = op.tile([P, chunk], F32, tag=f"o3_{t}")
                nc.vector.tensor_tensor(O3[:], O1[:], PP[:], mybir.AluOpType.add)
                nc.sync.dma_start(y_out[:, off:off + chunk], O3[:])
                off += chunk
                continue

            # product branches: factors w*(y +- D0) = SCALE_F*x + bias
            F3 = fp.tile([P, chunk * SIX], F32, tag="f3")
            nc.scalar.activation(F3[:], X[:], mybir.ActivationFunctionType.Copy,
                                 bias=BIAS_P, scale=SCALE_F)
            F4 = fp.tile([P, chunk * SIX], F32, tag="f4")
            nc.scalar.activation(F4[:], X[:], mybir.ActivationFunctionType.Copy,
                                 bias=BIAS_N, scale=SCALE_F)

            # level-1 pair products, written block-major: T[:, k*chunk+f]
            T3 = tp.tile([P, chunk * 3], F32, tag="t3")
            T3w = T3[:].rearrange("p (s f) -> p f s", s=3)
            F3v = F3[:].rearrange("p (f s) -> p f s", s=SIX)
            nc.vector.tensor_tensor(T3w, F3v[:, :, 0:3], F3v[:, :, 3:6],
                                    mybir.AluOpType.mult)
            T4 = tp.tile([P, chunk * 3], F32, tag="t4")
            T4w = T4[:].rearrange("p (s f) -> p f s", s=3)
            F4v = F4[:].rearrange("p (f s) -> p f s", s=SIX)
            nc.vector.tensor_tensor(T4w, F4v[:, :, 0:3], F4v[:, :, 3:6],
                                    mybir.AluOpType.mult)

            # levels 2-3 (contiguous block slices)
            V3 = vp.tile([P, chunk], F32, tag="v3")
            nc.vector.tensor_tensor(V3[:], T3[:, 0:chunk], T3[:, chunk:2 * chunk],
                                    mybir.AluOpType.mult)
            P3 = vp.tile([P, chunk], F32, tag="p3")
            nc.vector.tensor_tensor(P3[:], V3[:], T3[:, 2 * chunk:3 * chunk],
                                    mybir.AluOpType.mult)
            V4 = vp.tile([P, chunk], F32, tag=f"v4_{t}" if v4p4_pool else "v4")
            (nc.gpsimd if v4p4_pool else nc.vector).tensor_tensor(
                V4[:], T4[:, 0:chunk], T4[:, chunk:2 * chunk],
                                    mybir.AluOpType.mult)
            P4 = vp.tile([P, chunk], F32, tag=f"p4_{t}" if v4p4_pool else "p4")
            (nc.gpsimd if v4p4_pool else nc.vector).tensor_tensor(
                P4[:], V4[:], T4[:, 2 * chunk:3 * chunk],
                                    mybir.AluOpType.mult)

            # linear branch on gpsimd: L = sum_j x_j (tree), reading X directly
            # (X never carries WAR waits, and it keeps F3's readers DVE-only
            # so the ACT affines stay within their sync-wait budget)
            S1 = s1p.tile([P, chunk * 3], F32, tag=f"s1_{t}")
            S1w = S1[:].rearrange("p (s f) -> p f s", s=3)
            nc.gpsimd.tensor_tensor(S1w, Xv[:, :, 0:3], Xv[:, :, 3:6],
                                    mybir.AluOpType.add)
            S2 = slp.tile([P, chunk], F32, tag=f"s2_{t}")
            nc.gpsimd.tensor_tensor(S2[:], S1[:, 0:chunk], S1[:, chunk:2 * chunk],
                                    mybir.AluOpType.add)
            L = slp.tile([P, chunk], F32, tag=f"lsum{t}")
            nc.gpsimd.tensor_tensor(L[:], S2[:], S1[:, 2 * chunk:3 * chunk],
                                    mybir.AluOpType.add)

            if accum:
                # PP = P3 - P4 on DVE; O1 (linear part) written by ACT and
                # DMA'd as the base; PP accumulated into DRAM by SWDGE CCE.
                PP = vp.tile([P, chunk], F32, tag="ppd")
                nc.vector.tensor_tensor(PP[:], P3[:], P4[:],
                                        mybir.AluOpType.subtract)
                O1 = op.tile([P, chunk], F32, tag=f"o1_{t}")
                nc.scalar.activation(O1[:], L[:],
                                     mybir.ActivationFunctionType.Copy,
                                     bias=LIN_BIAS, scale=LIN_SCALE)
                nc.sync.dma_start(y_out[:, off:off + chunk], O1[:])
                nc.gpsimd.dma_start(y_out[:, off:off + chunk], PP[:],
                                    accum_op=mybir.AluOpType.add)
                off += chunk
                continue

            # combine: out = (LIN_SCALE*L + LIN_BIAS) + P3 - P4
            O1 = op.tile([P, chunk], F32, tag=f"o1_{t}")
            if o1_act:
                nc.scalar.activation(O1[:], L[:],
                                     mybir.ActivationFunctionType.Copy,
                                     bias=LIN_BIAS, scale=LIN_SCALE)
            else:
                nc.vector.tensor_scalar(O1[:], L[:], LIN_SCALE, LIN_BIAS,
                                        mybir.AluOpType.mult, mybir.AluOpType.add)
            O2 = op.tile([P, chunk], F32, tag=f"o2_{t}")
            (nc.gpsimd if o2_pool else nc.vector).tensor_tensor(
                O2[:], P3[:], P4[:], mybir.AluOpType.subtract)
            O3 = op.tile([P, chunk], F32, tag=f"o3_{t}")
            (nc.gpsimd if o3_pool else nc.vector).tensor_tensor(
                O3[:], O1[:], O2[:], mybir.AluOpType.add)

            nc.sync.dma_start(y_out[:, off:off + chunk], O3[:])
            off += chunk

    _split_sync_waits(nc)
    return nc


_NC_CACHE = None


def _get_nc():
    global _NC_CACHE
    if _NC_CACHE is None:
        _NC_CACHE = _build_bass()
    return _NC_CACHE


def kernel(inputs, lut=None, p_q_2_lut_table=None, **_unused):
    x = np.ascontiguousarray(np.asarray(inputs), dtype=np.float32)
    assert x.shape == (B, DEPTH, SIX), x.shape
    shards = x.reshape(N_CORES, P, FD_TOT * SIX)
    in_maps = [{"x": shards[i]} for i in range(N_CORES)]
    res = run_bass_kernel_spmd(_get_nc(), in_maps, list(range(N_CORES)))
    out = np.stack([res.results[i]["out"].reshape(-1) for i in range(N_CORES)])
    return out.reshape(B, DEPTH)



# revision 2
# speedup vs baseline: 1.3840x; 1.3840x over previous
"""Trainium2 Bass kernel for nn_LutLayer (B=512, depth=4096, SIX=6).

Math: per element with x = inputs[b, d, :] (6 values),
    out = C0 + C1 * sum_j y_j + S3 * [prod_j (y_j + D0) - prod_j (y_j - D0)]
with y_j = 2 x_j - 1 (closed form of the LUT mixture; see constants below).
|S3|^(1/6) is folded into the affine factors so all intermediates are O(1).

v2 layout strategy: inputs are shipped block-major per chunk
([j(6) x f(c)] blocks instead of interleaved [f x j]) so every on-chip
operand is a contiguous run.  All intermediates are fp16, which doubles
DVE tensor_tensor throughput (2x_1p) and quadruples tensor_scalar (4x_2p).
The B-branch product's sign is folded into its second-half factors
(F- for j>=3 is negated), so A - B becomes A + (-B): the whole combine
is an add chain and both branch products come from the same three
packed tensor_tensor ops.

Sharding: data-parallel over batch, 64 batches per core on 8 cores.
Output is written fp16 and widened to fp32 on the host.
"""

import sys
from contextlib import ExitStack

import numpy as np

if "/opt/trn_rl_repo" not in sys.path:
    sys.path.insert(0, "/opt/trn_rl_repo")

import concourse.bass as bass
import concourse.tile as tile
from concourse import mybir
from concourse.bass_utils import run_bass_kernel_spmd

N_CORES = 8
B, DEPTH, SIX = 512, 4096, 6
PER_CORE_B = B // N_CORES            # 64
N_ELEM = PER_CORE_B * DEPTH          # 262144 elements per core
P = 128                              # SBUF partitions
FD_TOT = N_ELEM // P                 # 2048 elements per partition
CHUNK = 512                          # elements per partition per chunk
N_CHUNKS = FD_TOT // CHUNK           # 4

# exact decomposition constants (fp64, derived offline; see module docstring)
D0 = 1.244957288028531
S3 = 0.020370985329978712
C1 = 0.33123508857995426
C0 = 1.0089040713978648e-11
W = S3 ** (1.0 / 6.0)                # folded branch weight, 0.52259911...

SCALE_F = float(2.0 * W)             # scale for both product branches
BIAS_P = float(W * (D0 - 1.0))       # bias for (y + D0) branch
BIAS_N = float(W * (-D0 - 1.0))      # bias for (y - D0) branch
DELTA = float(BIAS_N - BIAS_P)       # F- = F+ + DELTA
LIN_SCALE = float(2.0 * C1)          # applied to sum_j x_j
LIN_BIAS = float(C0 - 6.0 * C1)      # C0 + C1 * (-6)

F32 = mybir.dt.float32
F16 = mybir.dt.float16

# walrus codegen caps sync-wait commands per instruction (empirically: 1 for
# DMACopy and Pool/GPSIMD ops, 2 for ACT/DVE compute).  Tile's sem assignment
# can exceed that, so excess waits are split onto a standalone EventSemaphore
# on the same engine queue (program order makes that equivalent; the final
# all-engine barrier already uses 15-wait EventSemaphores, so they're legal).
_SPLIT_SKIP = {"InstEventSemaphore", "InstUnconditionalBranch",
               "InstCall", "InstRegisterMove"}


def _split_sync_waits(nc):
    for f in nc.m.functions:
        for b in f.blocks:
            new_insts = []
            for inst in b.instructions:
                si = inst.sync_info
                waits = list(si.on_wait) if si and si.on_wait else []
                budget = 1
                if type(inst).__name__ not in _SPLIT_SKIP and len(waits) > budget:
                    excess, keep = waits[:-budget], waits[-budget:]
                    for i in range(0, len(excess), 2):  # EventSemaphore: <=2 waits
                        ev = mybir.InstEventSemaphore(
                            name=f"{inst.name}-ws{i}",
                            opcode="EventSemaphore",
                            engine=inst.engine,
                            ins=[],
                            outs=[],
                            sync_info=mybir.SyncInfo(on_wait=excess[i:i + 2],
                                                     on_update=[]),
                            bass_nofuse=True,
                        )
                        new_insts.append(ev)
                    inst.sync_info = mybir.SyncInfo(on_wait=keep,
                                                    on_update=si.on_update)
                new_insts.append(inst)
            b.instructions = new_insts


def _build_bass(chunk=CHUNK):
    n_chunks = FD_TOT // chunk
    assert n_chunks * chunk == FD_TOT
    c = chunk
    nc = bass.Bass()
    # input: per chunk t, slab [t*6c:(t+1)*6c] holds j-major blocks
    # [j=0: f 0..c-1][j=1: ...] ... (block-major, host-prepped)
    x_in = nc.declare_dram_parameter("x", [P, FD_TOT * SIX], F32, isOutput=False)
    y_out = nc.declare_dram_parameter("out", [P, FD_TOT], F16, isOutput=True)

    with tile.TileContext(nc) as tc, ExitStack() as ctx:
        xp = ctx.enter_context(tc.tile_pool(name="x", bufs=1))
        fp = ctx.enter_context(tc.tile_pool(name="fct", bufs=2))
        tp = ctx.enter_context(tc.tile_pool(name="tv", bufs=1))
        sp = ctx.enter_context(tc.tile_pool(name="sum", bufs=1))
        lp = ctx.enter_context(tc.tile_pool(name="lin", bufs=1))
        op = ctx.enter_context(tc.tile_pool(name="out", bufs=1))
        for t in range(n_chunks):
            off = t * c
            # 1. load chunk: X [P, 6c] fp32, j-major blocks of c
            X = xp.tile([P, 6 * c], F32, tag=f"x{t}")
            nc.sync.dma_start(X[:], x_in[:, off * SIX:(off + c) * SIX])

            # 2. F [P, 12c] fp16: blocks 0:6 = F+ = S*x + b+ (ACT, contiguous)
            F = fp.tile([P, 12 * c], F16, tag="f")
            nc.scalar.activation(F[:, 0:6 * c], X[:],
                                 mybir.ActivationFunctionType.Copy,
                                 bias=BIAS_P, scale=SCALE_F)
            # 3./4. F- halves on DVE (fp16 tensor_scalar, 4x):
            #    blocks 6:9  =  F+[0:3] + DELTA       (j = 0..2)
            #    blocks 9:12 = -F+[3:6] - DELTA       (j = 3..5, sign-folded)
            nc.vector.tensor_scalar(F[:, 6 * c:9 * c], F[:, 0:3 * c],
                                    DELTA, None, mybir.AluOpType.add)
            nc.vector.tensor_scalar(F[:, 9 * c:12 * c], F[:, 3 * c:6 * c],
                                    -1.0, -DELTA,
                                    mybir.AluOpType.mult, mybir.AluOpType.add)

            # 5. pair products, one packed TT op:
            #    T[b,k] = F[b,k] * F[b,k+3]  (b=0: +branch, b=1: -branch)
            T = tp.tile([P, 6 * c], F16, tag="t")
            Fv = F[:].rearrange("p (b h m) -> p b h m", b=2, h=2)
            Tv = T[:].rearrange("p (b m) -> p b m", b=2)
            nc.vector.tensor_tensor(Tv, Fv[:, :, 0, :], Fv[:, :, 1, :],
                                    mybir.AluOpType.mult)

            # 6./7. product trees for both branches in lockstep
            Tk = T[:].rearrange("p (b k f) -> p b k f", b=2, k=3)
            V = tp.tile([P, 2 * c], F16, tag="v")
            Vv = V[:].rearrange("p (b f) -> p b f", b=2)
            nc.vector.tensor_tensor(Vv, Tk[:, :, 0, :], Tk[:, :, 1, :],
                                    mybir.AluOpType.mult)
            AB = tp.tile([P, 2 * c], F16, tag="ab")
            ABv = AB[:].rearrange("p (b f) -> p b f", b=2)
            nc.vector.tensor_tensor(ABv, Vv, Tk[:, :, 2, :],
                                    mybir.AluOpType.mult)
            # AB[0:c] = A = prod(F+ pairs), AB[c:2c] = -B

            # 8./9. linear branch on Pool: partial sums of x
            S1 = sp.tile([P, 3 * c], F16, tag=f"s1_{t}")
            Xv = X[:].rearrange("p (h g) -> p h g", h=2)
            nc.gpsimd.tensor_tensor(S1[:], Xv[:, 0, :], Xv[:, 1, :],
                                    mybir.AluOpType.add)
            S2 = sp.tile([P, c], F16, tag=f"s2_{t}")
            nc.gpsimd.tensor_tensor(S2[:], S1[:, 0:c], S1[:, c:2 * c],
                                    mybir.AluOpType.add)
            # 10. L = sum_j x_j (DVE fp16)
            L = lp.tile([P, c], F16, tag="l")
            nc.vector.tensor_tensor(L[:], S2[:], S1[:, 2 * c:3 * c],
                                    mybir.AluOpType.add)
            # 11. LP = LIN_SCALE * L + LIN_BIAS (ACT)
            LP = lp.tile([P, c], F16, tag=f"lp_{t}")
            nc.scalar.activation(LP[:], L[:],
                                 mybir.ActivationFunctionType.Copy,
                                 bias=LIN_BIAS, scale=LIN_SCALE)

            # 12./13. combine: out = A + (-B) + LP
            G = op.tile([P, c], F16, tag="g")
            nc.vector.tensor_tensor(G[:], AB[:, 0:c], AB[:, c:2 * c],
                                    mybir.AluOpType.add)
            O = op.tile([P, c], F16, tag=f"o_{t}")
            nc.vector.tensor_tensor(O[:], G[:], LP[:], mybir.AluOpType.add)

            # 14. store
            nc.sync.dma_start(y_out[:, off:off + c], O[:])

    _split_sync_waits(nc)
    return nc


_NC_CACHE = None


def _get_nc():
    global _NC_CACHE
    if _NC_CACHE is None:
        _NC_CACHE = _build_bass()
    return _NC_CACHE


def _make_in_maps(x):
    """x: (B, DEPTH, SIX) fp32 -> per-core block-major shards."""
    x = np.ascontiguousarray(np.asarray(x), dtype=np.float32)
    assert x.shape == (B, DEPTH, SIX), x.shape
    # per core: [P, n_chunks, chunk, 6] -> [P, n_chunks, 6, chunk]
    shards = np.ascontiguousarray(
        x.reshape(N_CORES, P, N_CHUNKS, CHUNK, SIX).transpose(0, 1, 2, 4, 3)
    ).reshape(N_CORES, P, FD_TOT * SIX)
    return [{"x": shards[i]} for i in range(N_CORES)]


def _postprocess(res):
    out = np.stack([np.asarray(res.results[i]["out"]).reshape(-1)
                    for i in range(N_CORES)])
    return out.astype(np.float32).reshape(B, DEPTH)


def kernel(inputs, lut=None, p_q_2_lut_table=None, **_unused):
    in_maps = _make_in_maps(inputs)
    res = run_bass_kernel_spmd(_get_nc(), in_maps, list(range(N_CORES)))
    return _postprocess(res)


# revision 3
# speedup vs baseline: 1.3944x; 1.0075x over previous
"""Trainium2 Bass kernel for nn_LutLayer (B=512, depth=4096, SIX=6).

Math: per element with x = inputs[b, d, :] (6 values),
    out = C0 + C1 * sum_j y_j + S3 * [prod_j (y_j + D0) - prod_j (y_j - D0)]
with y_j = 2 x_j - 1 (closed form of the LUT mixture; see constants below).
|S3|^(1/6) is folded into the affine factors u_j = S*x_j + b so all
intermediates are O(1).

v3 pipeline (per chunk, all intermediates fp16, every operand contiguous):
  ACT : F   = S*x + b            (fp16 factors of the + branch)
  Pool: PS  = x_j + x_{j+3}      (pair sums, reused twice below)
        S2  = PS0 + PS1
  DVE : T+  = F_j * F_{j+3}      (pair products, + branch)
        TD  = dS*PS + dc         (delta term: T- = T+ + d*(u_j+u_k) + d^2)
        T-  = T+ + TD            (pair products of the - branch, no F- tensor)
        V   = [T+0*T+1 | T-0*T-1]
        AB  = V * [T+2 | T-2]    (A and B)
        G   = A - B
        L   = S2 + PS2           (sum_j x_j)
        LP  = LIN_SCALE*L + LIN_BIAS
        O   = G + LP
fp16 doubles DVE tensor_tensor throughput (2x_1p) and quadruples
tensor_scalar (4x_2p).  Inputs are shipped block-major per chunk
([j(6) x f(c)] blocks) so every SBUF access is a contiguous run.

Sharding: data-parallel over batch, 64 batches per core on 8 cores.
Output is written fp16 and widened to fp32 on the host.
"""

import sys
from contextlib import ExitStack

import numpy as np

if "/opt/trn_rl_repo" not in sys.path:
    sys.path.insert(0, "/opt/trn_rl_repo")

import concourse.bass as bass
import concourse.tile as tile
from concourse import mybir
from concourse.bass_utils import run_bass_kernel_spmd

N_CORES = 8
B, DEPTH, SIX = 512, 4096, 6
PER_CORE_B = B // N_CORES            # 64
N_ELEM = PER_CORE_B * DEPTH          # 262144 elements per core
P = 128                              # SBUF partitions
FD_TOT = N_ELEM // P                 # 2048 elements per partition
CHUNKS = (256, 256, 512, 1024)       # ramp-in small, steady-state large
assert sum(CHUNKS) == FD_TOT

# exact decomposition constants (fp64, derived offline; see module docstring)
D0 = 1.244957288028531
S3 = 0.020370985329978712
C1 = 0.33123508857995426
C0 = 1.0089040713978648e-11
W = S3 ** (1.0 / 6.0)                # folded branch weight, 0.52259911...

SCALE_F = float(2.0 * W)             # scale for both product branches
BIAS_P = float(W * (D0 - 1.0))       # bias for (y + D0) branch
BIAS_N = float(W * (-D0 - 1.0))      # bias for (y - D0) branch
DELTA = float(BIAS_N - BIAS_P)       # u- = u+ + DELTA
TD_SCALE = float(DELTA * SCALE_F)    # TD = TD_SCALE*PS + TD_BIAS
TD_BIAS = float(2.0 * BIAS_P * DELTA + DELTA * DELTA)
LIN_SCALE = float(2.0 * C1)          # applied to sum_j x_j
LIN_BIAS = float(C0 - 6.0 * C1)      # C0 + C1 * (-6)

F32 = mybir.dt.float32
F16 = mybir.dt.float16

# walrus codegen caps sync-wait commands per instruction (empirically: 1 for
# DMACopy and Pool/GPSIMD ops, 2 for ACT/DVE compute).  Tile's sem assignment
# can exceed that, so excess waits are split onto a standalone EventSemaphore
# on the same engine queue (program order makes that equivalent; the final
# all-engine barrier already uses 15-wait EventSemaphores, so they're legal).
_SPLIT_SKIP = {"InstEventSemaphore", "InstUnconditionalBranch",
               "InstCall", "InstRegisterMove"}


def _split_sync_waits(nc):
    for f in nc.m.functions:
        for b in f.blocks:
            new_insts = []
            for inst in b.instructions:
                si = inst.sync_info
                waits = list(si.on_wait) if si and si.on_wait else []
                budget = 1
                if type(inst).__name__ not in _SPLIT_SKIP and len(waits) > budget:
                    excess, keep = waits[:-budget], waits[-budget:]
                    for i in range(0, len(excess), 2):  # EventSemaphore: <=2 waits
                        ev = mybir.InstEventSemaphore(
                            name=f"{inst.name}-ws{i}",
                            opcode="EventSemaphore",
                            engine=inst.engine,
                            ins=[],
                            outs=[],
                            sync_info=mybir.SyncInfo(on_wait=excess[i:i + 2],
                                                     on_update=[]),
                            bass_nofuse=True,
                        )
                        new_insts.append(ev)
                    inst.sync_info = mybir.SyncInfo(on_wait=keep,
                                                    on_update=si.on_update)
                new_insts.append(inst)
            b.instructions = new_insts


def _build_bass(chunks=CHUNKS):
    nc = bass.Bass()
    # input: per chunk t, the slab holds j-major blocks [j=0: f 0..c-1][j=1:...]
    x_in = nc.declare_dram_parameter("x", [P, FD_TOT * SIX], F32, isOutput=False)
    y_out = nc.declare_dram_parameter("out", [P, FD_TOT], F16, isOutput=True)

    with tile.TileContext(nc) as tc, ExitStack() as ctx:
        # every tile gets a per-chunk tag -> zero WAR dependencies anywhere
        pool = ctx.enter_context(tc.tile_pool(name="p", bufs=1))
        off = 0
        for t, c in enumerate(chunks):
            X = pool.tile([P, 6 * c], F32, tag=f"x{t}")
            nc.sync.dma_start(X[:], x_in[:, off * SIX:off * SIX + 6 * c])

            F = pool.tile([P, 6 * c], F16, tag=f"f{t}")
            nc.scalar.activation(F[:], X[:],
                                 mybir.ActivationFunctionType.Copy,
                                 bias=BIAS_P, scale=SCALE_F)

            PS = pool.tile([P, 3 * c], F16, tag=f"ps{t}")
            Xv = X[:].rearrange("p (h g) -> p h g", h=2)
            nc.gpsimd.tensor_tensor(PS[:], Xv[:, 0, :], Xv[:, 1, :],
                                    mybir.AluOpType.add)
            S2 = pool.tile([P, c], F16, tag=f"s2_{t}")
            nc.gpsimd.tensor_tensor(S2[:], PS[:, 0:c], PS[:, c:2 * c],
                                    mybir.AluOpType.add)

            # linear branch first: only Pool-dependent, starts before ACT ends
            L = pool.tile([P, c], F16, tag=f"l{t}")
            nc.vector.tensor_tensor(L[:], S2[:], PS[:, 2 * c:3 * c],
                                    mybir.AluOpType.add)
            LP = pool.tile([P, c], F16, tag=f"lp{t}")
            nc.vector.tensor_scalar(LP[:], L[:], LIN_SCALE, LIN_BIAS,
                                    mybir.AluOpType.mult, mybir.AluOpType.add)

            # product branches: T[0:3c] = +branch pairs, T[3c:6c] = -branch
            T = pool.tile([P, 6 * c], F16, tag=f"t{t}")
            nc.vector.tensor_tensor(T[:, 0:3 * c], F[:, 0:3 * c], F[:, 3 * c:6 * c],
                                    mybir.AluOpType.mult)
            TD = pool.tile([P, 3 * c], F16, tag=f"td{t}")
            nc.vector.tensor_scalar(TD[:], PS[:], TD_SCALE, TD_BIAS,
                                    mybir.AluOpType.mult, mybir.AluOpType.add)
            nc.vector.tensor_tensor(T[:, 3 * c:6 * c], T[:, 0:3 * c], TD[:],
                                    mybir.AluOpType.add)

            Tk = T[:].rearrange("p (b k f) -> p b k f", b=2, k=3)
            V = pool.tile([P, 2 * c], F16, tag=f"v{t}")
            Vv = V[:].rearrange("p (b f) -> p b f", b=2)
            nc.vector.tensor_tensor(Vv, Tk[:, :, 0, :], Tk[:, :, 1, :],
                                    mybir.AluOpType.mult)
            AB = pool.tile([P, 2 * c], F16, tag=f"ab{t}")
            ABv = AB[:].rearrange("p (b f) -> p b f", b=2)
            nc.vector.tensor_tensor(ABv, Vv, Tk[:, :, 2, :],
                                    mybir.AluOpType.mult)

            G = pool.tile([P, c], F16, tag=f"g{t}")
            nc.vector.tensor_tensor(G[:], AB[:, 0:c], AB[:, c:2 * c],
                                    mybir.AluOpType.subtract)
            O = pool.tile([P, c], F16, tag=f"o{t}")
            nc.vector.tensor_tensor(O[:], G[:], LP[:], mybir.AluOpType.add)

            nc.sync.dma_start(y_out[:, off:off + c], O[:])
            off += c

    _split_sync_waits(nc)
    return nc


_NC_CACHE = None


def _get_nc():
    global _NC_CACHE
    if _NC_CACHE is None:
        _NC_CACHE = _build_bass()
    return _NC_CACHE


def _make_in_maps(x):
    """x: (B, DEPTH, SIX) fp32 -> per-core block-major shards."""
    x = np.ascontiguousarray(np.asarray(x), dtype=np.float32)
    assert x.shape == (B, DEPTH, SIX), x.shape
    xs = x.reshape(N_CORES, P, FD_TOT, SIX)
    shards = np.empty((N_CORES, P, FD_TOT * SIX), dtype=np.float32)
    off = 0
    for c in CHUNKS:
        blk = xs[:, :, off:off + c, :].transpose(0, 1, 3, 2)  # [.., 6, c]
        shards[:, :, off * SIX:(off + c) * SIX] = blk.reshape(N_CORES, P, 6 * c)
        off += c
    return [{"x": shards[i]} for i in range(N_CORES)]


def _postprocess(res):
    out = np.stack([np.asarray(res.results[i]["out"]).reshape(-1)
                    for i in range(N_CORES)])
    return out.astype(np.float32).reshape(B, DEPTH)


def kernel(inputs, lut=None, p_q_2_lut_table=None, **_unused):
    in_maps = _make_in_maps(inputs)
    res = run_bass_kernel_spmd(_get_nc(), in_maps, list(range(N_CORES)))
    return _postprocess(res)
